# revision 13
# baseline (speedup 1.0000x reference)
"""Trainium2 Bass kernel for the FGWF objective:

    out = sum(cost_mat(graph, graph_b, prob, prob_b, tran, embedding, embedding_b) * tran)

Decomposition:
  sum(cost*T) = sum_i (f1_i + 0.5*||E_i||^2) * rowsum(T)_i
              + sum_j (f2_j + 0.5*||Eb_j||^2) * colsum(T)_j
              - 2 * <G @ T, T @ Gb>              (two 2048^3 matmuls, fp8 DoubleRow)
              - <E, T @ Eb>                      (feature cross term)
  with f1 = (G^2) @ p_s, f2 = (Gb^2) @ p_t.

Only the two 2048^3 bilinear terms run on device (they are the only O(N^3)
work); every rank-1 / feature term is O(N^2) and is evaluated on the host in
float64.  Device sharding over 8 NeuronCores: 2D grid (4 row-blocks x 2
col-blocks); each core computes <A, B> over its [512 x 1024] block via
64 + 64 fp8 DoubleRow matmuls and 8 DVE dot-reductions.  tran is pre-scaled
by 2^20 on the host so fp8-e4m3 avoids subnormal flush; the partial dots are
rescaled during the host-side reduction.
"""
import numpy as np
import ml_dtypes

import concourse.bass as bass
from concourse import mybir
from concourse import bass_utils
from concourse.tile import TileContext

BF16 = ml_dtypes.bfloat16
FP8 = ml_dtypes.float8_e4m3
F32 = mybir.dt.float32
BF = mybir.dt.bfloat16
F8 = mybir.dt.float8e4
TSCALE = 2.0 ** 20
N = 2048
NCORES = 8

_cache = {}


def _split_waits(nc):
    """The walrus build here rejects >1 sem-wait per instruction; hoist extra
    waits onto preceding same-engine nops."""
    MAXW = 1
    for fn in nc.m.functions:
        for b in fn.blocks:
            out = []
            changed = False
            for inst in b.instructions:
                si = inst.sync_info
                waits = list(si.on_wait) if si and si.on_wait else []
                if len(waits) > MAXW:
                    changed = True
                    k = 0
                    while len(waits) > MAXW:
                        chunk, waits = waits[:MAXW], waits[MAXW:]
                        out.append(mybir.InstNoOp(
                            name=f"{inst.name}-wsplit{k}", engine=inst.engine,
                            sync_info=mybir.SyncInfo(on_wait=chunk, on_update=[]),
                            bass_nofuse=True))
                        k += 1
                    inst.sync_info = mybir.SyncInfo(
                        on_wait=waits,
                        on_update=list(si.on_update) if si.on_update else [])
                out.append(inst)
            if changed:
                b.instructions = out


def _build():
    nc = bass.Bass()
    AL = mybir.AluOpType
    DR = mybir.MatmulPerfMode.DoubleRow

    # per-core inputs (host-sharded, fp8), pre-permuted to [128, 16, w] with
    # contraction index k = p*16 + t so every DMA chunk is contiguous per
    # partition (1-8KB lines)
    csT = nc.declare_dram_parameter("csT", [128, 16, 512], F8, isOutput=False)
    tcc = nc.declare_dram_parameter("tcc", [2, 128, 16, 512], F8, isOutput=False)
    tT = nc.declare_dram_parameter("tT", [128, 16, 512], F8, isOutput=False)
    gbc0 = nc.declare_dram_parameter("gbc0", [128, 16, 512], F8, isOutput=False)
    gbc1 = nc.declare_dram_parameter("gbc1", [128, 16, 512], F8, isOutput=False)

    # output: 8 pair-dot partials, one column per pair
    ocols_d = nc.declare_dram_parameter("out_cols", [128, 8], F32, isOutput=True)

    with TileContext(nc) as tc:
        with (
            tc.tile_pool(name="big", bufs=1) as big,
            tc.tile_pool(name="asb", bufs=8) as asb_p,
            tc.tile_pool(name="tout", bufs=2) as tout_p,
            tc.tile_pool(name="pa", bufs=1, space="PSUM") as pa_p,
            tc.tile_pool(name="pb", bufs=1, space="PSUM") as pb_p,
        ):
            csT_sb = big.tile([128, 16, 512], F8, tag="csT")
            tcc_sb = big.tile([128, 2, 16, 512], F8, tag="tcc")
            tT_sb = big.tile([128, 16, 512], F8, tag="tT")
            gbc0_sb = big.tile([128, 16, 512], F8, tag="gbc0")
            gbc1_sb = big.tile([128, 16, 512], F8, tag="gbc1")
            warm_sb = big.tile([128, 512], BF, tag="warm")
            ocols = big.tile([128, 8], F32, tag="ocols")

            # PE warmup: keep TensorE busy from the end of the preamble so the
            # HAM clock un-throttles before the data matmuls; results discarded.
            nc.vector.memset(warm_sb[:], 0.0)
            pw = pa_p.tile([128, 512], F32, name="pw", tag="paA0")
            for _ in range(7):
                nc.tensor.matmul(pw[:1, :], warm_sb[:, 0:1], warm_sb[:],
                                 start=True, stop=True, skip_group_check=True)

            # DMA issue order mirrors consumption order (A operands first in
            # small->large chunks) so neither the HW nor the scheduler's sim
            # ever has a later-phase matmul ready before an earlier one.
            ab_chunks = (slice(0, 1), slice(1, 2), slice(2, 4), slice(4, 8),
                         slice(8, 16))
            for ks in ab_chunks:
                nc.sync.dma_start(out=csT_sb[:, ks, :], in_=csT[:, ks, :])
                nc.gpsimd.dma_start(out=tcc_sb[:, 0, ks, :], in_=tcc[0, :, ks, :])
            nc.sync.dma_start(out=tcc_sb[:, 1, 0:8, :], in_=tcc[1, :, 0:8, :])
            nc.sync.dma_start(out=tcc_sb[:, 1, 8:16, :], in_=tcc[1, :, 8:16, :])
            nc.sync.dma_start(out=tT_sb[:, 0:8, :], in_=tT[:, 0:8, :])
            nc.sync.dma_start(out=gbc0_sb[:, 0:8, :], in_=gbc0[:, 0:8, :])
            nc.sync.dma_start(out=tT_sb[:, 8:16, :], in_=tT[:, 8:16, :])
            nc.sync.dma_start(out=gbc0_sb[:, 8:16, :], in_=gbc0[:, 8:16, :])
            nc.sync.dma_start(out=gbc1_sb[:, 0:8, :], in_=gbc1[:, 0:8, :])
            nc.sync.dma_start(out=gbc1_sb[:, 8:16, :], in_=gbc1[:, 8:16, :])

            # ---- A-phase: q-outer so each k-chunk is consumed as it lands ----
            paA = [pa_p.tile([128, 512], F32, name=f"paA{i}", tag=f"paA{i}") for i in range(4)]
            for q in range(8):
                for m in range(4):
                    msl = slice(128 * m, 128 * m + 128)
                    nc.tensor.matmul(paA[m][:], csT_sb[:, 2 * q:2 * q + 2, msl],
                                     tcc_sb[:, 0, 2 * q:2 * q + 2, :],
                                     start=(q == 0), stop=(q == 7),
                                     perf_mode=DR)
            paB = [pb_p.tile([128, 512], F32, name=f"paB{i}", tag=f"paB{i}") for i in range(4)]
            for q in range(8):
                for m in range(4):
                    msl = slice(128 * m, 128 * m + 128)
                    nc.tensor.matmul(paB[m][:], csT_sb[:, 2 * q:2 * q + 2, msl],
                                     tcc_sb[:, 1, 2 * q:2 * q + 2, :],
                                     start=(q == 0), stop=(q == 7),
                                     perf_mode=DR)

            # PSUM -> SBUF copies on ScalarE overlap the following matmuls
            a_tiles = {}
            for m in range(4):
                ca = asb_p.tile([128, 512], F32)
                nc.scalar.copy(ca[:], paA[m][:])
                a_tiles[m] = ca
            for m in range(4):
                cb = asb_p.tile([128, 512], F32)
                nc.scalar.copy(cb[:], paB[m][:])
                a_tiles[4 + m] = cb

            # ---- B + fused <A,B> ----
            # n=0: q-outer (chases the tT/gbc DMA stream); its dots run on DVE
            # during n=1.  n=1: m-outer so pair dots stagger and only the last
            # [128,512] dot remains after the final matmul.
            pbA = [pa_p.tile([128, 512], F32, name=f"pbA{i}", tag=f"paA{i}") for i in range(4)]
            for q in range(8):
                for m in range(4):
                    msl = slice(128 * m, 128 * m + 128)
                    nc.tensor.matmul(pbA[m][:], tT_sb[:, 2 * q:2 * q + 2, msl],
                                     gbc0_sb[:, 2 * q:2 * q + 2, :],
                                     start=(q == 0), stop=(q == 7),
                                     perf_mode=DR)
            for m in range(4):
                to = tout_p.tile([128, 512], F32)
                nc.vector.scalar_tensor_tensor(
                    out=to[:], in0=a_tiles[m][:], scalar=1.0, in1=pbA[m][:],
                    op0=AL.mult, op1=AL.mult,
                    accum_out=ocols[:, m:m + 1])

            for m in range(4):
                msl = slice(128 * m, 128 * m + 128)
                pbt = pb_p.tile([128, 512], F32, name=f"pbB{m}", tag=f"paB{m}")
                for q in range(8):
                    nc.tensor.matmul(pbt[:], tT_sb[:, 2 * q:2 * q + 2, msl],
                                     gbc1_sb[:, 2 * q:2 * q + 2, :],
                                     start=(q == 0), stop=(q == 7),
                                     perf_mode=DR)
                to = tout_p.tile([128, 512], F32)
                pair = 4 + m
                nc.vector.scalar_tensor_tensor(
                    out=to[:], in0=a_tiles[pair][:], scalar=1.0, in1=pbt[:],
                    op0=AL.mult, op1=AL.mult,
                    accum_out=ocols[:, pair:pair + 1])

            nc.sync.dma_start(out=ocols_d[:], in_=ocols[:])

    _split_waits(nc)
    return nc


def _prep_inputs(graph, embedding, prob, graph_b, embedding_b, prob_b, tran):
    G = np.asarray(graph, np.float32)
    GB = np.asarray(graph_b, np.float32)
    T = np.asarray(tran, np.float32)

    in_maps = []
    f8 = lambda x, shp: np.ascontiguousarray(x).astype(FP8).reshape(shp)
    for idx in range(NCORES):
        r, c = idx // 2, idx % 2
        rblk = slice(512 * r, 512 * r + 512)
        ccols = slice(1024 * c, 1024 * c + 1024)
        in_maps.append({
            "csT": f8(G[rblk, :].T, (128, 16, 512)),
            "tcc": f8(np.stack([T[:, 1024 * c:1024 * c + 512],
                                T[:, 1024 * c + 512:1024 * c + 1024]]) * TSCALE,
                      (2, 128, 16, 512)),
            "tT": f8(T[rblk, :].T * TSCALE, (128, 16, 512)),
            "gbc0": f8(GB[:, 1024 * c:1024 * c + 512], (128, 16, 512)),
            "gbc1": f8(GB[:, 1024 * c + 512:1024 * c + 1024], (128, 16, 512)),
        })
    return in_maps


def _host_terms(graph, embedding, prob, graph_b, embedding_b, prob_b, tran):
    """All O(N^2) terms of the objective, in f64 on the host."""
    G = np.asarray(graph, np.float32)
    E = np.asarray(embedding, np.float32)
    P = np.asarray(prob, np.float32).reshape(-1)
    GB = np.asarray(graph_b, np.float32)
    EB = np.asarray(embedding_b, np.float32)
    PB = np.asarray(prob_b, np.float32).reshape(-1)
    T = np.asarray(tran, np.float32)

    r = T.sum(axis=1, dtype=np.float64)            # rowsum(T)
    c = T.sum(axis=0, dtype=np.float64)            # colsum(T)
    f1 = ((G.astype(np.float64) ** 2) @ P.astype(np.float64))
    f2 = ((GB.astype(np.float64) ** 2) @ PB.astype(np.float64))
    nE = (E.astype(np.float64) ** 2).sum(axis=1)
    nEB = (EB.astype(np.float64) ** 2).sum(axis=1)
    S_emb = float((E.astype(np.float64) * (T.astype(np.float64) @
                                           EB.astype(np.float64))).sum())
    rank1 = float(((f1 + 0.5 * nE) * r).sum() + ((f2 + 0.5 * nEB) * c).sum())
    return rank1, S_emb


def _reduce(results, rank1, S_emb):
    S_main = 0.0
    for idx in range(NCORES):
        S_main += np.asarray(results[idx]["out_cols"], np.float64).sum()
    S_main /= TSCALE * TSCALE
    return np.float32(rank1 - 2.0 * S_main - S_emb)


def run_spmd(in_maps, trace=False, **kw):
    if "nc" not in _cache:
        _cache["nc"] = _build()
    return bass_utils.run_bass_kernel_spmd(
        _cache["nc"], in_maps, list(range(NCORES)), trace=trace, **kw)


def kernel(graph, embedding, prob, graph_b, embedding_b, prob_b, tran,
           weights, ole_coeff, idx):
    in_maps = _prep_inputs(graph, embedding, prob, graph_b, embedding_b,
                           prob_b, tran)
    rank1, S_emb = _host_terms(graph, embedding, prob, graph_b, embedding_b,
                               prob_b, tran)
    last_err = None
    for _attempt in range(3):
        try:
            res = run_spmd(in_maps)
            return _reduce(res.results, rank1, S_emb)
        except Exception as e:  # transient NRT device errors seen under axon
            last_err = e
    raise last_err


# revision 14
# speedup vs baseline: 1.0816x; 1.0816x over previous
"""Trainium2 Bass kernel for the FGWF objective:

    out = sum(cost_mat(graph, graph_b, prob, prob_b, tran, embedding, embedding_b) * tran)

Decomposition:
  sum(cost*T) = sum_i (f1_i + 0.5*||E_i||^2) * rowsum(T)_i
              + sum_j (f2_j + 0.5*||Eb_j||^2) * colsum(T)_j
              - 2 * <G @ T, T @ Gb>              (two 2048^3 matmuls, fp8 DoubleRow)
              - <E, T @ Eb>                      (feature cross term)
  with f1 = (G^2) @ p_s, f2 = (Gb^2) @ p_t.

Only the two 2048^3 bilinear terms run on device (they are the only O(N^3)
work); every rank-1 / feature term is O(N^2) and is evaluated on the host in
float64.  Device sharding over 8 NeuronCores: 2D grid (4 row-blocks x 2
col-blocks); each core computes <A, B> over its [512 x 1024] block via
64 + 64 fp8 DoubleRow matmuls and 8 DVE dot-reductions.  tran is pre-scaled
by 2^20 on the host so fp8-e4m3 avoids subnormal flush; the partial dots are
rescaled during the host-side reduction.
"""
import numpy as np
import ml_dtypes

import concourse.bass as bass
from concourse import mybir
from concourse import bass_utils
from concourse.tile import TileContext

BF16 = ml_dtypes.bfloat16
FP8 = ml_dtypes.float8_e4m3
F32 = mybir.dt.float32
BF = mybir.dt.bfloat16
F8 = mybir.dt.float8e4
TSCALE = 2.0 ** 20
N = 2048
NCORES = 8

_cache = {}


def _split_waits(nc):
    """The walrus build here rejects >1 sem-wait per instruction; hoist extra
    waits onto preceding same-engine nops."""
    MAXW = 1
    for fn in nc.m.functions:
        for b in fn.blocks:
            out = []
            changed = False
            for inst in b.instructions:
                si = inst.sync_info
                waits = list(si.on_wait) if si and si.on_wait else []
                if len(waits) > MAXW:
                    changed = True
                    k = 0
                    while len(waits) > MAXW:
                        chunk, waits = waits[:MAXW], waits[MAXW:]
                        out.append(mybir.InstNoOp(
                            name=f"{inst.name}-wsplit{k}", engine=inst.engine,
                            sync_info=mybir.SyncInfo(on_wait=chunk, on_update=[]),
                            bass_nofuse=True))
                        k += 1
                    inst.sync_info = mybir.SyncInfo(
                        on_wait=waits,
                        on_update=list(si.on_update) if si.on_update else [])
                out.append(inst)
            if changed:
                b.instructions = out


def _build():
    nc = bass.Bass()
    AL = mybir.AluOpType
    DR = mybir.MatmulPerfMode.DoubleRow

    # per-core inputs (host-sharded, fp8), pre-permuted to [128, 16, w] with
    # contraction index k = p*16 + t so every DMA chunk is contiguous per
    # partition (1-8KB lines)
    csT = nc.declare_dram_parameter("csT", [128, 16, 512], F8, isOutput=False)
    tcc = nc.declare_dram_parameter("tcc", [2, 128, 16, 512], F8, isOutput=False)
    tT = nc.declare_dram_parameter("tT", [128, 16, 512], F8, isOutput=False)
    gbc0 = nc.declare_dram_parameter("gbc0", [128, 16, 512], F8, isOutput=False)
    gbc1 = nc.declare_dram_parameter("gbc1", [128, 16, 512], F8, isOutput=False)

    # output: 8 pair-dot partials, one column per pair
    ocols_d = nc.declare_dram_parameter("out_cols", [128, 8], F32, isOutput=True)

    with TileContext(nc) as tc:
        with (
            tc.tile_pool(name="big", bufs=1) as big,
            tc.tile_pool(name="asb", bufs=8) as asb_p,
            tc.tile_pool(name="tout", bufs=2) as tout_p,
            tc.tile_pool(name="pa", bufs=1, space="PSUM") as pa_p,
            tc.tile_pool(name="pb", bufs=1, space="PSUM") as pb_p,
        ):
            csT_sb = big.tile([128, 16, 512], F8, tag="csT")
            tcc_sb = big.tile([128, 2, 16, 512], F8, tag="tcc")
            tT_sb = big.tile([128, 16, 512], F8, tag="tT")
            gbc0_sb = big.tile([128, 16, 512], F8, tag="gbc0")
            gbc1_sb = big.tile([128, 16, 512], F8, tag="gbc1")
            warm_sb = big.tile([128, 512], BF, tag="warm")
            ocols = big.tile([128, 8], F32, tag="ocols")

            # PE warmup: keep TensorE busy from the end of the preamble so the
            # HAM clock un-throttles before the data matmuls; results discarded.
            nc.vector.memset(warm_sb[:], 0.0)
            pw = pa_p.tile([128, 512], F32, name="pw", tag="paA0")
            for _ in range(7):
                nc.tensor.matmul(pw[:1, :], warm_sb[:, 0:1], warm_sb[:],
                                 start=True, stop=True, skip_group_check=True)

            # DMA issue order mirrors consumption order (A operands first in
            # small->large chunks) so neither the HW nor the scheduler's sim
            # ever has a later-phase matmul ready before an earlier one.
            ab_chunks = (slice(0, 1), slice(1, 2), slice(2, 4), slice(4, 6),
                         slice(6, 10), slice(10, 16))
            for ks in ab_chunks:
                nc.sync.dma_start(out=csT_sb[:, ks, :], in_=csT[:, ks, :])
                nc.sync.dma_start(out=tcc_sb[:, 0, ks, :], in_=tcc[0, :, ks, :])
            nc.sync.dma_start(out=tcc_sb[:, 1, 0:8, :], in_=tcc[1, :, 0:8, :])
            nc.sync.dma_start(out=tcc_sb[:, 1, 8:16, :], in_=tcc[1, :, 8:16, :])
            nc.sync.dma_start(out=tT_sb[:, 0:8, :], in_=tT[:, 0:8, :])
            nc.sync.dma_start(out=gbc0_sb[:, 0:8, :], in_=gbc0[:, 0:8, :])
            nc.sync.dma_start(out=tT_sb[:, 8:16, :], in_=tT[:, 8:16, :])
            nc.sync.dma_start(out=gbc0_sb[:, 8:16, :], in_=gbc0[:, 8:16, :])
            nc.sync.dma_start(out=gbc1_sb[:, 0:8, :], in_=gbc1[:, 0:8, :])
            nc.sync.dma_start(out=gbc1_sb[:, 8:16, :], in_=gbc1[:, 8:16, :])

            # ---- A-phase: q-outer so each k-chunk is consumed as it lands ----
            paA = [pa_p.tile([128, 512], F32, name=f"paA{i}", tag=f"paA{i}") for i in range(4)]
            for q in range(8):
                for m in range(4):
                    msl = slice(128 * m, 128 * m + 128)
                    nc.tensor.matmul(paA[m][:], csT_sb[:, 2 * q:2 * q + 2, msl],
                                     tcc_sb[:, 0, 2 * q:2 * q + 2, :],
                                     start=(q == 0), stop=(q == 7),
                                     perf_mode=DR)
            paB = [pb_p.tile([128, 512], F32, name=f"paB{i}", tag=f"paB{i}") for i in range(4)]
            for q in range(8):
                for m in range(4):
                    msl = slice(128 * m, 128 * m + 128)
                    nc.tensor.matmul(paB[m][:], csT_sb[:, 2 * q:2 * q + 2, msl],
                                     tcc_sb[:, 1, 2 * q:2 * q + 2, :],
                                     start=(q == 0), stop=(q == 7),
                                     perf_mode=DR)

            # PSUM -> SBUF copies on ScalarE overlap the following matmuls
            a_tiles = {}
            for m in range(4):
                ca = asb_p.tile([128, 512], F32)
                nc.scalar.copy(ca[:], paA[m][:])
                a_tiles[m] = ca
            for m in range(4):
                cb = asb_p.tile([128, 512], F32)
                nc.scalar.copy(cb[:], paB[m][:])
                a_tiles[4 + m] = cb

            # ---- B + fused <A,B> ----
            # n=0: q-outer (chases the tT/gbc DMA stream); its dots run on DVE
            # during n=1.  n=1: m-outer so pair dots stagger and only the last
            # [128,512] dot remains after the final matmul.
            pbA = [pa_p.tile([128, 512], F32, name=f"pbA{i}", tag=f"paA{i}") for i in range(4)]
            for q in range(8):
                for m in range(4):
                    msl = slice(128 * m, 128 * m + 128)
                    nc.tensor.matmul(pbA[m][:], tT_sb[:, 2 * q:2 * q + 2, msl],
                                     gbc0_sb[:, 2 * q:2 * q + 2, :],
                                     start=(q == 0), stop=(q == 7),
                                     perf_mode=DR)
            for m in range(4):
                to = tout_p.tile([128, 512], F32)
                nc.vector.scalar_tensor_tensor(
                    out=to[:], in0=a_tiles[m][:], scalar=1.0, in1=pbA[m][:],
                    op0=AL.mult, op1=AL.mult,
                    accum_out=ocols[:, m:m + 1])

            for m in range(4):
                msl = slice(128 * m, 128 * m + 128)
                pbt = pb_p.tile([128, 512], F32, name=f"pbB{m}", tag=f"paB{m}")
                for q in range(8):
                    nc.tensor.matmul(pbt[:], tT_sb[:, 2 * q:2 * q + 2, msl],
                                     gbc1_sb[:, 2 * q:2 * q + 2, :],
                                     start=(q == 0), stop=(q == 7),
                                     perf_mode=DR)
                to = tout_p.tile([128, 512], F32)
                pair = 4 + m
                nc.vector.scalar_tensor_tensor(
                    out=to[:], in0=a_tiles[pair][:], scalar=1.0, in1=pbt[:],
                    op0=AL.mult, op1=AL.mult,
                    accum_out=ocols[:, pair:pair + 1])

            nc.sync.dma_start(out=ocols_d[:], in_=ocols[:])

    _split_waits(nc)
    return nc


def _prep_inputs(graph, embedding, prob, graph_b, embedding_b, prob_b, tran):
    G = np.asarray(graph, np.float32)
    GB = np.asarray(graph_b, np.float32)
    T = np.asarray(tran, np.float32)

    in_maps = []
    f8 = lambda x, shp: np.ascontiguousarray(x).astype(FP8).reshape(shp)
    for idx in range(NCORES):
        r, c = idx // 2, idx % 2
        rblk = slice(512 * r, 512 * r + 512)
        ccols = slice(1024 * c, 1024 * c + 1024)
        in_maps.append({
            "csT": f8(G[rblk, :].T, (128, 16, 512)),
            "tcc": f8(np.stack([T[:, 1024 * c:1024 * c + 512],
                                T[:, 1024 * c + 512:1024 * c + 1024]]) * TSCALE,
                      (2, 128, 16, 512)),
            "tT": f8(T[rblk, :].T * TSCALE, (128, 16, 512)),
            "gbc0": f8(GB[:, 1024 * c:1024 * c + 512], (128, 16, 512)),
            "gbc1": f8(GB[:, 1024 * c + 512:1024 * c + 1024], (128, 16, 512)),
        })
    return in_maps


def _host_terms(graph, embedding, prob, graph_b, embedding_b, prob_b, tran):
    """All O(N^2) terms of the objective, in f64 on the host."""
    G = np.asarray(graph, np.float32)
    E = np.asarray(embedding, np.float32)
    P = np.asarray(prob, np.float32).reshape(-1)
    GB = np.asarray(graph_b, np.float32)
    EB = np.asarray(embedding_b, np.float32)
    PB = np.asarray(prob_b, np.float32).reshape(-1)
    T = np.asarray(tran, np.float32)

    r = T.sum(axis=1, dtype=np.float64)            # rowsum(T)
    c = T.sum(axis=0, dtype=np.float64)            # colsum(T)
    f1 = ((G.astype(np.float64) ** 2) @ P.astype(np.float64))
    f2 = ((GB.astype(np.float64) ** 2) @ PB.astype(np.float64))
    nE = (E.astype(np.float64) ** 2).sum(axis=1)
    nEB = (EB.astype(np.float64) ** 2).sum(axis=1)
    S_emb = float((E.astype(np.float64) * (T.astype(np.float64) @
                                           EB.astype(np.float64))).sum())
    rank1 = float(((f1 + 0.5 * nE) * r).sum() + ((f2 + 0.5 * nEB) * c).sum())
    return rank1, S_emb


def _reduce(results, rank1, S_emb):
    S_main = 0.0
    for idx in range(NCORES):
        S_main += np.asarray(results[idx]["out_cols"], np.float64).sum()
    S_main /= TSCALE * TSCALE
    return np.float32(rank1 - 2.0 * S_main - S_emb)


def run_spmd(in_maps, trace=False, **kw):
    if "nc" not in _cache:
        _cache["nc"] = _build()
    return bass_utils.run_bass_kernel_spmd(
        _cache["nc"], in_maps, list(range(NCORES)), trace=trace, **kw)


def kernel(graph, embedding, prob, graph_b, embedding_b, prob_b, tran,
           weights, ole_coeff, idx):
    in_maps = _prep_inputs(graph, embedding, prob, graph_b, embedding_b,
                           prob_b, tran)
    rank1, S_emb = _host_terms(graph, embedding, prob, graph_b, embedding_b,
                               prob_b, tran)
    last_err = None
    for _attempt in range(3):
        try:
            res = run_spmd(in_maps)
            return _reduce(res.results, rank1, S_emb)
        except Exception as e:  # transient NRT device errors seen under axon
            last_err = e
    raise last_err
